# revision 26
# baseline (speedup 1.0000x reference)
"""BERT-CRF loss kernel for Trainium2 (8 NeuronCores, data-parallel over positions).

Math: loss = sum_b(forward_b - cumsum(gold)_b) for a CRF whose forward scan runs
over the flattened B*S steps (batch carryover).  The log-semiring scan is
reassociated into per-chunk (L=4 positions) transfer matrices computed on
device in scaled probability space:

  feats[pos,t] = hidden @ W.T + b     (PE, fp8, pos-major)
  m[pos]       = max over live tags   (Pool reduce)
  fsub         = feats - m            (Pool sub, f32, shipped for gold score)
  EF           = exp(fsub)            (ACT, bf16)
  chunk scan: At <- (Eblk.T @ At) * EF_s   (PE matmul + DVE j-major bcast mul)

Positions are column-permuted on the host so that each scan step reads a
contiguous 32-column EF slice, and the 8 sentences (groups) are pair-packed
at partition offsets {0,16} inside 32-aligned slots (PE transposes write
[32,128] blocks at legal partition starts).

Host combines the 8192 chunk matrices (f64, tree per sentence + sequential
sentence carry) and computes the gold score from the shipped fsub + m.
"""
import numpy as np
import ml_dtypes
from contextlib import ExitStack

import concourse.bass as bass
import concourse.mybir as mybir
from concourse.tile import TileContext
from concourse.tile_rust import add_dep_helper
from concourse.bass_utils import run_bass_kernel_spmd

B, S, H, T = 64, 512, 768, 12
START, STOP, NEG = 10, 11, -10000.0
L = 4                      # chunk length (positions per transfer matrix)
NCORES = 8
P_CORE = B * S // NCORES   # 4096 positions per core
G = 8                      # sentences (groups) per core
KPG = S // L               # 128 chunks per group
NQ = 4                     # quarter chains
CPQ = KPG // NQ            # 32 chunks per group per quarter
NLIVE = 10

FP8NP = ml_dtypes.float8_e4m3fn
BF16 = ml_dtypes.bfloat16

F32 = mybir.dt.float32
BF = mybir.dt.bfloat16
FP8 = mybir.dt.float8e4

# device column <-> original position permutation (per core)
# position q = g*512 + k*L + s ; Q = k//CPQ ; c = k%CPQ
# col = Q*1024 + g*128 + s*32 + c
_cols = np.arange(P_CORE)
_Q = _cols // 1024
_g = (_cols % 1024) // 128
_s = (_cols % 128) // 32
_c = _cols % 32
PERM = (_g * S + (_Q * CPQ + _c) * L + _s)   # PERM[col] = original position


def _build_nc():
    nc = bass.Bass()
    hidT = nc.declare_dram_parameter("hidT", [H, P_CORE], FP8, isOutput=False)
    cf8 = nc.declare_dram_parameter("cf8", [128, 212], FP8, isOutput=False)
    cb16 = nc.declare_dram_parameter("cb16", [128, 576], BF, isOutput=False)
    fm_out = nc.declare_dram_parameter("fm_out", [128, 416], BF, isOutput=True)
    a_out = nc.declare_dram_parameter("a_out", [128, NQ * 320], BF, isOutput=True)

    with ExitStack() as ctx:
        tc = ctx.enter_context(TileContext(nc))
        const_pool = ctx.enter_context(tc.tile_pool(name="const", bufs=1))
        hid_pool = ctx.enter_context(tc.tile_pool(name="hid", bufs=12))
        sb_pool = ctx.enter_context(tc.tile_pool(name="sb", bufs=1))
        at_pool = ctx.enter_context(tc.tile_pool(name="at", bufs=8))
        fp_pool = ctx.enter_context(tc.tile_pool(name="fps", bufs=2, space="PSUM"))
        eft_pool = ctx.enter_context(tc.tile_pool(name="eftp", bufs=1, space="PSUM"))
        ps_pool = ctx.enter_context(tc.tile_pool(name="pss", bufs=4, space="PSUM"))
        warm_pool = ctx.enter_context(tc.tile_pool(name="warm", bufs=1, space="PSUM"))

        # ---- persistent SBUF tiles ----
        cf8_sb = const_pool.tile([128, 212], FP8)
        cb16_sb = const_pool.tile([128, 576], BF)
        efpos = sb_pool.tile([128, 512], BF)       # pos-major EF, pair-packed
        ef_sb = sb_pool.tile([128, 512], BF)       # tag-major EF (scan operand)
        fraws = [sb_pool.tile([128, 96], BF, name=f"fraw{q}")
                 for q in range(NQ)]               # feats psum copied to SBUF
        fm_sb = sb_pool.tile([128, 416], BF)       # m (cols 0:32) | fsub (32:416)
        a_sb = sb_pool.tile([128, NQ * 320], BF)   # final chunk matrices

        eft_ps = eft_pool.tile([128, 512], BF)     # transposed EF psum

        # ---- pad init: zero the 4-wide pad columns of efpos (cols g*16+12..16)
        memset_i = nc.gpsimd.memset(
            bass.AP(efpos.tensor, efpos[:, 12:16].offset,
                    [efpos[:, :].ap[0], [16, 32], [1, 4]]),
            0.0,
        )

        # ---- input DMAs, split across the three DMA-capable queues ----
        # half h covers cols h*2048:(h+1)*2048 ; row block hs covers 128 h-rows
        hid = {}
        def hdma(eng, hs, h):
            t = hid_pool.tile([128, 2048], FP8, name=f"hid_{hs}_{h}", tag="hid")
            eng.dma_start(out=t[:, :],
                          in_=hidT[hs * 128:(hs + 1) * 128, h * 2048:(h + 1) * 2048])
            hid[(hs, h)] = t

        # Pool: 5 hid ; SP: 5 hid + cb16 ; ACT: cf8 + 2 hid
        nc.scalar.dma_start(out=cf8_sb[:, :], in_=cf8[:, :])
        hdma(nc.gpsimd, 0, 0)
        hdma(nc.sync, 3, 0)
        hdma(nc.gpsimd, 1, 0)
        hdma(nc.sync, 4, 0)
        hdma(nc.scalar, 5, 0)
        hdma(nc.gpsimd, 2, 0)
        nc.sync.dma_start(out=cb16_sb[:, :], in_=cb16[:, :])
        hdma(nc.sync, 2, 1)
        hdma(nc.gpsimd, 0, 1)
        hdma(nc.sync, 3, 1)
        hdma(nc.gpsimd, 1, 1)
        hdma(nc.sync, 4, 1)
        hdma(nc.scalar, 5, 1)

        ident = cb16_sb[:, 0:128]
        eblk = cb16_sb[:, 128:256]
        e40jm = cb16_sb[:, 256:576]

        # ---- warmups: ramp PE p-state, load ACT exp table early, and absorb
        # the cf8/cb16/memset sems into the PE clock so later PE instructions
        # need at most one sync wait.
        wp = warm_pool.tile([128, 384], F32)
        scr = const_pool.tile([1, 16], BF)
        nc.scalar.activation(scr[0:1, 0:8], cf8_sb[0:1, 0:8],
                             mybir.ActivationFunctionType.Exp)
        for _ in range(3):
            nc.tensor.matmul(wp[:, 0:212], lhsT=cf8_sb[:, 0:128],
                             rhs=cf8_sb[:, 0:212], start=True, stop=True)
        wm = nc.tensor.matmul(wp[:, 0:128], lhsT=cb16_sb[:, 0:128],
                              rhs=cb16_sb[:, 0:128], start=True, stop=True)
        add_dep_helper(wm.ins, memset_i.ins, True, "absorb memset sem into PE")
        scrp = const_pool.tile([1, 8], BF)

        def feats_quarter(Q):
            h = Q // 2
            fp_ps = fp_pool.tile([128, 96], F32, name=f"fp{Q}", tag="fp")
            if Q >= 2:
                # absorber pair for the recycled feats-psum bank: ab0 pulls the
                # ACT (copy) sem into the PE clock; ab1 (dummy first-writer)
                # carries the bank-reuse hazard wait.
                ab0 = nc.tensor.matmul(wp[0:1, 0:1], lhsT=fraws[Q - 2][0:1, 0:1],
                                       rhs=fraws[Q - 2][0:1, 0:1],
                                       start=True, stop=True)
                ab1 = nc.tensor.matmul(fp_ps[0:1, 0:1], lhsT=cf8_sb[0:1, 0:1],
                                       rhs=cf8_sb[0:1, 0:1], start=True,
                                       stop=True, skip_group_check=True)
                add_dep_helper(ab1.ins, ab0.ins, False, "absorber ordering")
            for g in range(G):
                blk = fp_ps[:, g * 12:(g + 1) * 12]
                for hs in range(6):
                    nc.tensor.matmul(
                        blk,
                        lhsT=hid[(hs, h)][:, (Q % 2) * 1024 + g * 128:
                                          (Q % 2) * 1024 + (g + 1) * 128],
                        rhs=cf8_sb[:, hs * 12:(hs + 1) * 12],
                        start=(hs == 0), stop=False,
                        skip_group_check=True,
                    )
                nc.tensor.matmul(
                    blk, lhsT=cf8_sb[0:1, 72:200], rhs=cf8_sb[0:1, 200:212],
                    start=False, stop=True, skip_group_check=True,
                )
            # copy feats psum -> SBUF bf16 (ACT; gpsimd cannot touch PSUM)
            fraw = fraws[Q]
            cp_i = nc.scalar.activation(
                fraw[:, :], fp_ps[:, :],
                mybir.ActivationFunctionType.Copy,
            )
            # absorber: pull the ACT-copy sem into the Pool clock so the sub
            # below only needs the DVE (reduce) wait.  (A real data-dependent
            # instruction — NoOp waits are not credited by the wait-elision
            # pass since NoOps may be fused away.)
            nc.gpsimd.tensor_copy(scrp[0:1, 2 * Q:2 * Q + 2],
                                  fraw[0:1, 0:2])
            fq3 = fraw[:, :].rearrange("p (b j) -> p b j", j=12)
            # m = max over live tags (DVE, bf16 2x)
            nc.vector.reduce_max(
                out=fm_sb[:, Q * 8:(Q + 1) * 8],
                in_=bass.AP(fq3.tensor, fq3.offset,
                            [fq3.ap[0], fq3.ap[1], [1, NLIVE]]),
                axis=mybir.AxisListType.X,
            )
            # fsub = feats - m (Pool, all SBUF)
            msl = fm_sb[:, Q * 8:(Q + 1) * 8]
            m_b = bass.AP(msl.tensor, msl.offset,
                          [msl.ap[0], msl.ap[1], [0, 12]])
            nc.gpsimd.tensor_sub(
                fm_sb[:, 32 + Q * 96:32 + (Q + 1) * 96]
                .rearrange("p (b j) -> p b j", j=12),
                fq3, m_b,
            )
            # EF = exp(fsub) into pair-packed layout (ACT)
            eo = efpos[:, Q * 128:(Q + 1) * 128]
            eo3 = bass.AP(eo.tensor, eo.offset, [eo.ap[0], [16, 8], [1, 12]])
            nc.scalar.activation(
                eo3,
                fm_sb[:, 32 + Q * 96:32 + (Q + 1) * 96]
                .rearrange("p (b j) -> p b j", j=12),
                mybir.ActivationFunctionType.Exp,
            )
            # transpose pairs into tag-major psum
            for v in range(4):
                nc.tensor.transpose(
                    eft_ps[32 * v:32 * v + 32, Q * 128:(Q + 1) * 128],
                    efpos[:, Q * 128 + v * 32:Q * 128 + v * 32 + 32],
                    ident,
                    tile_position=(0, 32 * v),
                )
            # copy psum -> sbuf (DVE, 2x)
            nc.vector.tensor_copy(ef_sb[:, Q * 128:(Q + 1) * 128],
                                  eft_ps[:, Q * 128:(Q + 1) * 128])

        def ef_ap(Q, s):
            base = ef_sb[:, Q * 128 + s * 32:Q * 128 + s * 32 + 32]
            return bass.AP(base.tensor, base.offset,
                           [base.ap[0], [0, NLIVE], base.ap[1]])

        def scan_quarter(Q):
            if Q == 0:
                # absorb the cb16 (e40jm) DMA-queue sem into the DVE clock
                nc.vector.tensor_copy(scrp[0:1, 6:8], cb16_sb[0:1, 256:258])
            at = at_pool.tile([128, 320], BF, name=f"at_{Q}_0", tag="at")
            nc.vector.tensor_mul(
                at[:, :].rearrange("p (j c) -> p j c", c=32),
                e40jm.rearrange("p (j c) -> p j c", c=32),
                ef_ap(Q, 0),
            )
            for s in range(1, L):
                # absorber 1: pull the DVE (At ready) sem into the PE clock
                ab1 = nc.tensor.matmul(wp[0:1, 0:1], lhsT=at[0:1, 0:1],
                                       rhs=at[0:1, 0:1], start=True, stop=True)
                ps = ps_pool.tile([128, 320], F32)
                # absorber 2: dummy first-writer carries the PSUM bank-reuse
                # hazard wait
                ab2 = nc.tensor.matmul(ps[0:1, 0:1], lhsT=cb16_sb[0:1, 0:1],
                                       rhs=cb16_sb[0:1, 0:1], start=True,
                                       stop=True, skip_group_check=True)
                add_dep_helper(ab2.ins, ab1.ins, False, "absorber ordering")
                nc.tensor.matmul(ps[:, :], lhsT=eblk, rhs=at[:, :],
                                 start=True, stop=True, skip_group_check=True)
                if s < L - 1:
                    at2 = at_pool.tile([128, 320], BF, name=f"at_{Q}_{s}", tag="at")
                    out_ap = at2[:, :].rearrange("p (j c) -> p j c", c=32)
                else:
                    at2 = None
                    sl = a_sb[:, Q * 320:(Q + 1) * 320]
                    out_ap = sl.rearrange("p (j c) -> p j c", c=32)
                nc.vector.tensor_mul(
                    out_ap,
                    ps[:, :].rearrange("p (j c) -> p j c", c=32),
                    ef_ap(Q, s),
                )
                at = at2

        for Q in range(NQ):
            feats_quarter(Q)
            scan_quarter(Q)
            if Q == 1:
                nc.gpsimd.dma_start(out=a_out[:, 0:640], in_=a_sb[:, 0:640])
        nc.gpsimd.dma_start(out=fm_out[:, :], in_=fm_sb[:, :])
        nc.gpsimd.dma_start(out=a_out[:, 640:1280], in_=a_sb[:, 640:1280])
    return nc


_NC_CACHE = None


def _get_nc():
    global _NC_CACHE
    if _NC_CACHE is None:
        _NC_CACHE = _build_nc()
    return _NC_CACHE


def _build_consts(W, b, transitions):
    E = np.exp(transitions.astype(np.float64))
    E[START, :] = 0.0
    E[STOP, :] = 0.0
    E[:, STOP] = 0.0
    E = E.astype(np.float32)

    cf8 = np.zeros((128, 212), np.float32)
    # wt: cf8[p, hs*12+t] = W[t, hs*128+p]
    cf8[:, 0:72] = W.T.reshape(6, 128, T).transpose(1, 0, 2).reshape(128, 72)
    cf8[0, 72:200] = 1.0
    cf8[0, 200:212] = b
    cf8 = cf8.astype(FP8NP)

    cb16 = np.zeros((128, 576), np.float32)
    cb16[:, 0:128] = np.eye(128)
    # eblk: eblk[32v+off+j, 32v+off+i] = E[i, j]  (live 10x10)
    for v in range(4):
        for off in (0, 16):
            o = 32 * v + off
            cb16[o:o + NLIVE, 128 + o:128 + o + NLIVE] = E[:NLIVE, :NLIVE].T
    # e40jm: e40jm[32v+off+i, j*32+c] = E[i, j]
    blk = np.zeros((32, 320), np.float32)
    for off in (0, 16):
        for i in range(NLIVE):
            for j in range(NLIVE):
                blk[off + i, j * 32:(j + 1) * 32] = E[i, j]
    for v in range(4):
        cb16[32 * v:32 * v + 32, 256:576] = blk
    cb16 = cb16.astype(BF16)
    return cf8, cb16


def _run_device(hidden, W, b, transitions, trace=False, tmpdir=None):
    cf8, cb16 = _build_consts(W, b, transitions)
    flat = hidden.reshape(B * S, H)
    in_maps = []
    for core in range(NCORES):
        blk = flat[core * P_CORE:(core + 1) * P_CORE]        # [4096, 768]
        hT = np.ascontiguousarray(blk[PERM].T).astype(FP8NP)  # [768, 4096]
        in_maps.append({"hidT": hT, "cf8": cf8, "cb16": cb16})
    return run_bass_kernel_spmd(
        _get_nc(), in_maps, list(range(NCORES)), trace=trace, tmpdir=tmpdir)


def _logsumexp(x, axis):
    mx = np.max(x, axis=axis)
    mx_safe = np.where(np.isfinite(mx), mx, 0.0)
    out = mx + np.log(np.sum(np.exp(x - np.expand_dims(mx_safe, axis)), axis=axis))
    return np.where(np.isfinite(mx), out, -np.inf)


def _host_combine(results, transitions, tags):
    trans = transitions.astype(np.float64)
    err = np.errstate(invalid="ignore", divide="ignore", over="ignore")
    err.__enter__()

    # unpack fm_out: m and fsub in device (permuted) order -> original order
    feats = np.zeros((NCORES, P_CORE, T), np.float64)
    m_all = np.zeros((NCORES, P_CORE), np.float64)
    for core, r in enumerate(results):
        fm = np.asarray(r["fm_out"]).astype(np.float64)   # [128, 416]
        # device col layout: col = Q*1024 + g*128 + s*32 + c ; tile t=Q*8+g,
        # partition p = s*32+c ; m = fm[:, t], fsub = fm[:, 32+12t+j]
        m_dev = fm[:, 0:32]            # [p, t]
        fs_dev = fm[:, 32:416].reshape(128, 32, 12)   # [p, t, j]
        # device column index for (t, p): col = (t//8)*1024 + (t%8)*128 + p
        tt = np.arange(32)
        pp = np.arange(128)
        colidx = (tt[None, :] // 8) * 1024 + (tt[None, :] % 8) * 128 + pp[:, None]
        q = PERM[colidx]               # [p, t] original position
        m_all[core, q] = m_dev
        feats[core, q, :] = fs_dev + m_dev[:, :, None]

    feats = feats.reshape(B, S, T)     # [B, S, T] (= fsub + m, exact device feats)
    m_all = m_all.reshape(B, S)

    # unpack chunk matrices: A[b, k][i, j] (live 10x10), log + scale
    logA = np.zeros((B, KPG, NLIVE, NLIVE), np.float64)
    for core, r in enumerate(results):
        a = np.asarray(r["a_out"]).astype(np.float64)     # [128, 1280]
        a4 = a.reshape(128, NQ, NLIVE, 32)                 # [p, Q, j, c]
        for g in range(G):
            rows = 32 * (g // 2) + 16 * (g % 2)
            blkm = a4[rows:rows + NLIVE]                   # [i, Q, j, c]
            logA[core * G + g] = np.log(blkm).transpose(1, 3, 0, 2).reshape(
                KPG, NLIVE, NLIVE)
    scale = m_all.reshape(B, KPG, L).sum(axis=2)           # [B, KPG]
    logA = logA + scale[:, :, None, None]

    # first global chunk: explicit recurrence from init (full 12-state)
    v0 = np.full(T, NEG, np.float64)
    v0[START] = 0.0
    for s in range(L):
        v0 = _logsumexp(trans[None, :, :] + v0[None, None, :], axis=2)[0] \
            + feats[0, s]
    # replace chunk (0,0) with identity in the tree
    ident = np.full((NLIVE, NLIVE), -np.inf)
    np.fill_diagonal(ident, 0.0)
    logA[0, 0] = ident

    # tree-combine the 128 chunk mats of each sentence -> one mat per sentence
    mats = logA.reshape(B * KPG, NLIVE, NLIVE)
    n = B * KPG
    while n > B:
        A2 = mats[0::2]        # earlier chunk
        B2 = mats[1::2]        # later chunk
        x = B2[:, :, :, None] + A2[:, None, :, :]          # [n/2, i, j, k]
        mats = _logsumexp(x, axis=2)
        n //= 2

    # sequential carry across sentences
    last = np.zeros((B, T), np.float64)
    v = v0.copy()
    for b in range(B):
        if b == 0:
            vl = _logsumexp(mats[0] + v[None, :NLIVE], axis=1)
        else:
            vl = _logsumexp(mats[b] + v[None, :NLIVE], axis=1)
        v = np.concatenate([vl, [-np.inf, -np.inf]])
        last[b] = v
        if b + 1 < B:
            # start of next sentence: nothing special (carryover CRF)
            pass
    forward_score = _logsumexp(last + trans[STOP][None, :], axis=1)
    err.__exit__(None, None, None)

    tags = np.asarray(tags)
    tags_ext = np.concatenate(
        [np.full((B, 1), START, dtype=tags.dtype), tags], axis=1)
    prev, nxt = tags_ext[:, :-1], tags_ext[:, 1:]
    trans_sc = trans[nxt, prev].sum(axis=1)
    emit_sc = np.take_along_axis(
        feats, nxt[..., None].astype(np.int64), axis=2)[..., 0].sum(axis=1)
    gold = trans_sc + emit_sc + trans[STOP, tags_ext[:, -1]]
    gold_cum = np.cumsum(gold)
    out = np.sum(forward_score - gold_cum)
    return np.array([out], dtype=np.float32)


def kernel(hidden, W, b, transitions, tags, _trace=False, _tmpdir=None):
    hidden = np.asarray(hidden, dtype=np.float32)
    W = np.asarray(W, dtype=np.float32)
    b = np.asarray(b, dtype=np.float32)
    transitions = np.asarray(transitions, dtype=np.float32)
    res = _run_device(hidden, W, b, transitions, trace=_trace, tmpdir=_tmpdir)
    out = _host_combine(res.results, transitions, tags)
    if _trace:
        return out, res
    return out
